# revision 23
# baseline (speedup 1.0000x reference)
"""Trainium2 Bass kernel for nn_Attention_60155311948227 (sparse_attention).

Sharding: data-parallel over batch B=8 across the 8 NeuronCores (1 sample per
core); the four FC weights are replicated (each core DMAs its own copy).

Per-core pipeline (GEMMs in bf16 with fp32 PSUM accumulation):
  XCT  = x_context^T   fp32 HWDGE row-strips -> PE transpose -> bf16
  A^T  = sum_{7x7}(x)  flat-layout loads, DVE reduce, PE transpose
  K^T  = BN(relu(kW @ xc^T + b))   [d1(part), m] bf16, kept in SBUF
  kn2  = ones-matmul of K^T**2 -> rk = 1/||k_row||;  qn2 -> rq
  S    = (Q^T)^T @ K^T  [n, m] * rq (row) * rk (col bcast) + amask, softmax
  P^T  = PE transpose;  P^T rows scaled by rv (V-row norms)
  V^T  -> PE transpose -> V_nat [m(part), d2] bf16 (unnormalized)
  WV^T = V_nat/P^T contraction; F^T = BN(relu(fW @ WV + b)) fp32
  out  = x + F broadcast over 7x7 (flat-layout passes, F via DRAM bounce)

Weights stream as fp32 column-strips on HWDGE and are cast to bf16 on-chip
(ACT/DVE) — the SWDGE cast-DMA path measures only ~45 GB/s aggregate.
"""

import sys

import numpy as np

try:
    import concourse.bacc as bacc
except ImportError:  # pragma: no cover
    sys.path.insert(0, "/opt/trn_rl_repo")
    import concourse.bacc as bacc

import ml_dtypes

import concourse.bass as bass
import concourse.tile as tile
from concourse import mybir
from concourse import bass_utils
from concourse.masks import make_identity

F32 = mybir.dt.float32
BF16 = mybir.dt.bfloat16
AF = mybir.ActivationFunctionType
ALU = mybir.AluOpType
AX = mybir.AxisListType

BN_EPS = 1e-5
NEG_MASK = -50.0
TEMP_INV = 100.0
NORM_EPS = 1e-24

FULL = dict(B=8, n=64, m=2048, D0=1024, C0=2048, D1=2048, D2=2048, KK=49)

P = 128


def build_program(cfg=None, num_devices=8):
    """Emit the SPMD per-core Bass program. Returns the compiled Bacc."""
    cfg = dict(FULL if cfg is None else cfg)
    n, m, D0, C0, D1, D2, KK = (
        cfg["n"], cfg["m"], cfg["D0"], cfg["C0"], cfg["D1"], cfg["D2"], cfg["KK"]
    )
    nc_d0, nc_c0, nc_d1, nc_d2, nc_m = D0 // P, C0 // P, D1 // P, D2 // P, m // P
    n_nt = max(1, m // 512)          # 512-wide moving-dim tiles
    NT = m // n_nt
    inv_kk = 1.0 / KK
    mh = m // 2
    # flat x/out chunking: partition p = (n, dhalf); per-partition contiguous
    DQ = 32                          # D-rows per flat chunk
    FD = DQ * KK                     # flat chunk free size
    NFC = (D0 // 2) // DQ            # number of flat chunks (8)

    nc = bacc.Bacc("TRN2", target_bir_lowering=False, debug=False,
                   num_devices=num_devices)

    def din(name, shape, dt=F32):
        return nc.dram_tensor(name, shape, dt, kind="ExternalInput").ap()

    x_in = din("x", [n, D0, KK])
    xc_in = din("xc", [m, C0])
    wqt = din("wqt", [D0, D1])
    wkt = din("wkt", [C0, D1])
    wvt = din("wvt", [C0, D2])
    wft = din("wft", [D2, D0])
    amask = din("amask", [m], BF16)
    qcb = din("qcb", [P, nc_d1]); qcg = din("qcg", [P, nc_d1]); qc2 = din("qc2", [P, nc_d1])
    kcb = din("kcb", [P, nc_d1]); kcg = din("kcg", [P, nc_d1]); kc2 = din("kc2", [P, nc_d1])
    vcb = din("vcb", [P, nc_d2]); vcg = din("vcg", [P, nc_d2]); vc2 = din("vc2", [P, nc_d2])
    fcb = din("fcb", [P, nc_d0]); fcg = din("fcg", [P, nc_d0]); fc2 = din("fc2", [P, nc_d0])
    out_d = nc.dram_tensor("out", [n, D0, KK], F32, kind="ExternalOutput").ap()
    x_flat = x_in.rearrange("nn d k -> (nn d k)").rearrange(
        "(p f) -> p f", p=P)          # [128, D0*KK/2] per-partition contiguous
    out_flat = out_d.rearrange("nn d k -> (nn d k)").rearrange(
        "(p f) -> p f", p=P)

    with tile.TileContext(nc) as tc:
        with (
            tc.tile_pool(name="consts", bufs=1) as consts,
            tc.tile_pool(name="bigmat", bufs=1) as bigmat,
            tc.tile_pool(name="w8", bufs=2) as w8,          # fp32 strips (8KB)
            tc.tile_pool(name="strips", bufs=2) as strips,  # bf16 strips (4KB)
            tc.tile_pool(name="smalls", bufs=2) as smalls,
            tc.tile_pool(name="wides", bufs=1) as wides,
            tc.tile_pool(name="xpool", bufs=2) as xpool,
            tc.tile_pool(name="ps", bufs=1, space="PSUM") as ps,
            tc.tile_pool(name="dscr", bufs=1, space="DRAM") as dscr,
        ):
            # ---------------- constants ----------------
            ident = consts.tile([P, P], BF16)
            make_identity(nc, ident)
            ident32 = consts.tile([P, P], F32)
            make_identity(nc, ident32)
            ones_col = consts.tile([P, 1], BF16)
            nc.vector.memset(ones_col, 1.0)
            eps_col = consts.tile([P, 1], F32)
            nc.vector.memset(eps_col, NORM_EPS)

            def cload(ap_in, nch):
                t = consts.tile([P, nch], F32, name=f"c_{ap_in.tensor.name}")
                nc.sync.dma_start(out=t, in_=ap_in)
                return t

            qcb_t = cload(qcb, nc_d1); qcg_t = cload(qcg, nc_d1); qc2_t = cload(qc2, nc_d1)
            kcb_t = cload(kcb, nc_d1); kcg_t = cload(kcg, nc_d1); kc2_t = cload(kc2, nc_d1)
            vcb_t = cload(vcb, nc_d2); vcg_t = cload(vcg, nc_d2); vc2_t = cload(vc2, nc_d2)
            fcb_t = cload(fcb, nc_d0); fcg_t = cload(fcg, nc_d0); fc2_t = cload(fc2, nc_d0)

            amask_bc = consts.tile([n, m], BF16, tag="amask_bc")
            nc.gpsimd.dma_start(
                out=amask_bc,
                in_=bass.AP(tensor=amask.tensor, offset=amask.offset,
                            ap=[[0, n]] + list(amask.ap)),
            )

            # ---------------- XCT: transpose x_context ----------------
            # contiguous fp32 row-strips; 16 fp32 PE transposes per strip into
            # an 8KB PSUM tile (alternating tag A/B); ACT copy casts to bf16.
            xct = bigmat.tile([P, nc_c0, m], BF16, tag="xct")
            for i in range(nc_m):
                xcs = w8.tile([P, C0], F32, tag="w8", name="xcs")
                nc.sync.dma_start(out=xcs, in_=xc_in[i * P:(i + 1) * P, :])
                tpx = ps.tile([P, nc_c0, P], F32,
                              tag=("A" if i % 2 == 0 else "B"), name="tpx")
                for c in range(nc_c0):
                    nc.tensor.transpose(tpx[:, c, :], xcs[:, c * P:(c + 1) * P],
                                        ident32)
                nc.scalar.copy(out=xct[:, :, i * P:(i + 1) * P], in_=tpx)

            # ---------------- pooling: A^T = sum_k x (flat layout) ----------
            at = consts.tile([P, nc_d0, n], BF16)
            for g in range(NFC):
                xt = xpool.tile([P, DQ, KK], F32, tag="x", name="xt")
                nc.sync.dma_start(out=xt,
                                  in_=x_flat[:, g * FD:(g + 1) * FD])
                asum = smalls.tile([P, DQ], F32, name="asum")
                nc.vector.reduce_sum(asum, xt, axis=AX.X)
                atp = ps.tile([DQ, P], F32, tag="B", name="atp")
                nc.tensor.transpose(atp, asum, ident32)
                # columns p=(nn, dhalf); D row = dhalf*D0/2 + g*DQ + dd2
                for half in range(2):
                    dglob = half * (D0 // 2) + g * DQ
                    base = dglob % P
                    nc.vector.tensor_copy(
                        out=at[base:base + DQ, dglob // P, :],
                        in_=atp[:, half::2])

            # ---------------- K^T projection (kept in SBUF) ----------------
            def wstrip(w_ap, j, ncc, name):
                """fp32 column-strip [P, ncc, P] -> cast to bf16 on-chip."""
                wf = w8.tile([P, ncc, P], F32, tag="w8", name=f"{name}f")
                nc.scalar.dma_start(
                    out=wf,
                    in_=w_ap[:, j * P:(j + 1) * P].rearrange(
                        "(c p) w -> p c w", p=P))
                wb = strips.tile([P, ncc, P], BF16, tag="strip", name=f"{name}b")
                if j % 2 == 0:
                    nc.vector.tensor_copy(out=wb, in_=wf)
                else:
                    nc.scalar.copy(out=wb, in_=wf)
                return wb

            kt = bigmat.tile([P, nc_d1, m], BF16, tag="ktv", name="kt")
            kn2 = ps.tile([1, m], F32, tag="B")
            for j in range(nc_d1):
                kws = wstrip(wkt, j, nc_c0, "kws")
                kp = ps.tile([P, m], F32, tag="A", name="kp")
                for c in range(nc_c0):
                    for nt in range(n_nt):
                        nc.tensor.matmul(kp[:, nt * NT:(nt + 1) * NT],
                                         kws[:, c, :],
                                         xct[:, c, nt * NT:(nt + 1) * NT],
                                         start=(c == 0), stop=(c == nc_c0 - 1))
                ktj = kt[:, j, :]
                nc.scalar.activation(ktj[:, :mh], kp[:, :mh], AF.Relu,
                                     bias=kcb_t[:, j:j + 1])
                nc.vector.tensor_scalar(out=ktj[:, mh:], in0=kp[:, mh:],
                                        scalar1=kcb_t[:, j:j + 1], scalar2=0.0,
                                        op0=ALU.add, op1=ALU.max)
                nc.vector.tensor_scalar(out=ktj, in0=ktj,
                                        scalar1=kcg_t[:, j:j + 1],
                                        scalar2=kc2_t[:, j:j + 1],
                                        op0=ALU.mult, op1=ALU.add)
                ksq = w8.tile([P, m], BF16, tag="w8", name="ksq")
                nc.scalar.activation(ksq, ktj, AF.Square)
                for nt in range(n_nt):
                    nc.tensor.matmul(kn2[:, nt * NT:(nt + 1) * NT], ones_col,
                                     ksq[:, nt * NT:(nt + 1) * NT],
                                     start=(j == 0), stop=(j == nc_d1 - 1))
            # rk chain: sqrt -> scatter [P, m/P] -> recip -> DRAM -> bcast
            rk_row = smalls.tile([1, m], F32, name="rk_row")
            nc.scalar.activation(rk_row, kn2, AF.Sqrt, bias=eps_col[:1, :])
            scr_k = dscr.tile([m], F32, name="scr_k")
            nc.gpsimd.dma_start(out=scr_k, in_=rk_row)
            rk128 = smalls.tile([P, nc_m], F32, name="rk128")
            nc.gpsimd.dma_start(out=rk128,
                                in_=bass.AP(tensor=scr_k.tensor, offset=scr_k.offset,
                                            ap=[[1, P], [P, nc_m]]))
            nc.vector.reciprocal(rk128, rk128)
            scr_k2 = dscr.tile([m], F32, name="scr_k2")
            nc.gpsimd.dma_start(
                out=bass.AP(tensor=scr_k2.tensor, offset=scr_k2.offset,
                            ap=[[1, P], [P, nc_m]]),
                in_=rk128)
            rk_bc = wides.tile([n, m], F32, name="rk_bc", tag="rk_bc")
            nc.gpsimd.dma_start(out=rk_bc,
                                in_=bass.AP(tensor=scr_k2.tensor, offset=scr_k2.offset,
                                            ap=[[0, n], [1, m]]))

            # ---------------- Q^T projection ----------------
            qt = consts.tile([P, nc_d1, n], BF16)
            qn2 = ps.tile([1, n], F32, tag="B")
            for j in range(nc_d1):
                qws = wstrip(wqt, j, nc_d0, "qws")
                qp = ps.tile([P, n], F32, tag="A", name="qp")
                for c in range(nc_d0):
                    nc.tensor.matmul(qp, qws[:, c, :], at[:, c, :],
                                     start=(c == 0), stop=(c == nc_d0 - 1))
                q1 = smalls.tile([P, n], BF16, name="q1")
                nc.scalar.activation(q1, qp, AF.Relu, bias=qcb_t[:, j:j + 1],
                                     scale=inv_kk)
                nc.vector.tensor_scalar(out=qt[:, j, :], in0=q1,
                                        scalar1=qcg_t[:, j:j + 1],
                                        scalar2=qc2_t[:, j:j + 1],
                                        op0=ALU.mult, op1=ALU.add)
                qsq = smalls.tile([P, n], BF16, name="qsq")
                nc.scalar.activation(qsq, qt[:, j, :], AF.Square)
                nc.tensor.matmul(qn2, ones_col, qsq,
                                 start=(j == 0), stop=(j == nc_d1 - 1))
            rq_row = smalls.tile([1, n], F32, name="rq_row")
            nc.scalar.activation(rq_row, qn2, AF.Sqrt, bias=eps_col[:1, :])
            scr_q = dscr.tile([n], F32, name="scr_q")
            nc.gpsimd.dma_start(out=scr_q, in_=rq_row)
            rq_col = smalls.tile([n, 1], F32, name="rq_col")
            nc.gpsimd.dma_start(out=rq_col,
                                in_=bass.AP(tensor=scr_q.tensor, offset=scr_q.offset,
                                            ap=[[1, n], [1, 1]]))
            nc.vector.reciprocal(rq_col, rq_col)

            # ---------------- S = Q K^T, softmax ----------------
            sp = ps.tile([n, m], F32, tag="B", name="sp")
            for j in range(nc_d1):
                for nt in range(n_nt):
                    nc.tensor.matmul(sp[:, nt * NT:(nt + 1) * NT], qt[:, j, :],
                                     kt[:, j, nt * NT:(nt + 1) * NT],
                                     start=(j == 0), stop=(j == nc_d1 - 1))
            nc.vector.tensor_scalar(out=sp, in0=sp, scalar1=rq_col,
                                    scalar2=None, op0=ALU.mult)
            nc.vector.tensor_mul(sp, sp, rk_bc)
            nc.vector.tensor_add(sp, sp, amask_bc)
            mxn = smalls.tile([n, 1], F32, name="mxn")
            nc.vector.tensor_reduce(mxn, sp, axis=AX.X, op=ALU.max, negate=True)
            ebias = smalls.tile([n, 1], F32, name="ebias")
            nc.vector.tensor_scalar_mul(ebias, mxn, TEMP_INV)
            p_t = consts.tile([n, m], BF16, name="p_t", tag="amask_bc")
            pden = smalls.tile([n, 1], F32, name="pden")
            nc.scalar.activation(p_t, sp, AF.Exp, bias=ebias, scale=TEMP_INV,
                                 accum_out=pden)
            nc.vector.reciprocal(pden, pden)
            nc.vector.tensor_scalar_mul(p_t, p_t, pden)
            ptp = ps.tile([P, nc_m, n], BF16, tag="B", name="ptp")
            for i in range(nc_m):
                nc.tensor.transpose(ptp[:, i, :], p_t[:, i * P:(i + 1) * P],
                                    ident[:n, :n])
            pt_sb = consts.tile([P, nc_m, n], BF16)
            nc.vector.tensor_copy(out=pt_sb, in_=ptp)

            # ---------------- V^T -> V_nat (unnormalized) ----------------
            v_nat = bigmat.tile([P, nc_m, D2], BF16, tag="ktv", name="v_nat")
            for j in range(nc_d2):
                vws = wstrip(wvt, j, nc_c0, "vws")
                vp = ps.tile([P, m], F32, tag="A", name="vp")
                for c in range(nc_c0):
                    for nt in range(n_nt):
                        nc.tensor.matmul(vp[:, nt * NT:(nt + 1) * NT],
                                         vws[:, c, :],
                                         xct[:, c, nt * NT:(nt + 1) * NT],
                                         start=(c == 0), stop=(c == nc_c0 - 1))
                vtj = strips.tile([P, m], BF16, tag="strip", name="vtj")
                nc.scalar.activation(vtj[:, :mh], vp[:, :mh], AF.Relu,
                                     bias=vcb_t[:, j:j + 1])
                nc.vector.tensor_scalar(out=vtj[:, mh:], in0=vp[:, mh:],
                                        scalar1=vcb_t[:, j:j + 1], scalar2=0.0,
                                        op0=ALU.add, op1=ALU.max)
                nc.vector.tensor_scalar(out=vtj, in0=vtj,
                                        scalar1=vcg_t[:, j:j + 1],
                                        scalar2=vc2_t[:, j:j + 1],
                                        op0=ALU.mult, op1=ALU.add)
                vtp = ps.tile([P, nc_m, P], BF16, tag="B", name="vtp")
                for i in range(nc_m):
                    nc.tensor.transpose(vtp[:, i, :], vtj[:, i * P:(i + 1) * P],
                                        ident)
                nc.vector.tensor_copy(out=v_nat[:, :, j * P:(j + 1) * P],
                                      in_=vtp)
            # rv = 1/||v_row||; folded into P^T rows (per-partition there)
            for i in range(nc_m):
                vsq = w8.tile([P, D2], BF16, tag="w8", name="vsq")
                vn2 = smalls.tile([P, 1], F32, name="vn2")
                nc.scalar.activation(vsq, v_nat[:, i, :], AF.Square,
                                     accum_out=vn2)
                rv = smalls.tile([P, 1], F32, name="rv")
                nc.scalar.activation(rv, vn2, AF.Sqrt, bias=eps_col)
                nc.vector.reciprocal(rv, rv)
                nc.vector.tensor_scalar_mul(pt_sb[:, i, :], pt_sb[:, i, :], rv)

            # ---------------- WV^T = sum_i V_nat_i^T P^T_i ----------------
            wvt_sb = consts.tile([P, nc_d2, n], BF16)
            for j in range(nc_d2):
                wvp = ps.tile([P, n], F32, tag="A", name="wvp")
                for i in range(nc_m):
                    nc.tensor.matmul(wvp, v_nat[:, i, j * P:(j + 1) * P],
                                     pt_sb[:, i, :],
                                     start=(i == 0), stop=(i == nc_m - 1))
                nc.vector.tensor_copy(out=wvt_sb[:, j, :], in_=wvp)

            # ---------------- F^T projection (fp32) ----------------
            ft = consts.tile([P, nc_d0, n], F32)
            for dd in range(nc_d0):
                fws = wstrip(wft, dd, nc_d2, "fws")
                fp = ps.tile([P, n], F32, tag="A", name="fp")
                for j in range(nc_d2):
                    nc.tensor.matmul(fp, fws[:, j, :], wvt_sb[:, j, :],
                                     start=(j == 0), stop=(j == nc_d2 - 1))
                f1 = smalls.tile([P, n], F32, name="f1")
                nc.scalar.activation(f1, fp, AF.Relu, bias=fcb_t[:, dd:dd + 1])
                nc.vector.tensor_scalar(out=ft[:, dd, :], in0=f1,
                                        scalar1=fcg_t[:, dd:dd + 1],
                                        scalar2=fc2_t[:, dd:dd + 1],
                                        op0=ALU.mult, op1=ALU.add)

            # ---------------- out = x + F (flat layout) ----------------
            # F^T -> F_nat (PE transposes) -> DRAM bounce -> [(n dhalf), D0/2]
            fnat = wides.tile([n, D0], F32, tag="rk_bc")
            for dd in range(nc_d0):
                ftp = ps.tile([n, P], F32, tag="B", name="ftp")
                nc.tensor.transpose(ftp, ft[:, dd, :], ident32)
                nc.vector.tensor_copy(out=fnat[:, dd * P:(dd + 1) * P], in_=ftp)
            f_scr = dscr.tile([n, D0], F32, name="f_scr")
            nc.sync.dma_start(out=f_scr, in_=fnat)
            fperm = wides.tile([P, D0 // 2], F32, name="fperm", tag="rk_bc")
            nc.sync.dma_start(
                out=fperm,
                in_=bass.AP(tensor=f_scr.tensor, offset=f_scr.offset,
                            ap=[[D0, n], [D0 // 2, 2], [1, D0 // 2]]))
            for g in range(NFC):
                xo = xpool.tile([P, DQ, KK], F32, tag="x", name="xo")
                nc.sync.dma_start(out=xo, in_=x_flat[:, g * FD:(g + 1) * FD])
                nc.vector.tensor_add(
                    xo, xo,
                    fperm[:, g * DQ:(g + 1) * DQ].unsqueeze(2)
                    .broadcast_to([P, DQ, KK]))
                nc.scalar.dma_start(out=out_flat[:, g * FD:(g + 1) * FD], in_=xo)

    nc.compile()
    return nc


_CACHED = {}
# test-harness hook: extra kwargs for run_bass_kernel_spmd (e.g. trace=True)
_RUN_KWARGS = {}


def _get_program():
    if "nc" not in _CACHED:
        _CACHED["nc"] = build_program()
    return _CACHED["nc"]


def _bn_consts(b, gamma, beta, mean, var, nch):
    g = (gamma / np.sqrt(var + BN_EPS)).astype(np.float32)
    b2 = (beta - g * mean).astype(np.float32)
    def fold(v):
        return np.ascontiguousarray(np.asarray(v, np.float32).reshape(nch, P).T)
    return fold(b), fold(g), fold(b2)


def kernel(**inputs):
    cfg = FULL
    B, n, m = cfg["B"], cfg["n"], cfg["m"]
    D0, C0, D1, D2, KK = cfg["D0"], cfg["C0"], cfg["D1"], cfg["D2"], cfg["KK"]

    x = np.asarray(inputs["x"], dtype=np.float32).reshape(B, n, D0, KK)
    xc = np.asarray(inputs["x_context"], dtype=np.float32)
    nvalid = np.asarray(inputs["num_valid_context_items"]).reshape(B).astype(np.int64)

    wqt = np.ascontiguousarray(np.asarray(inputs["q_W"], np.float32).T)
    wkt = np.ascontiguousarray(np.asarray(inputs["k_W"], np.float32).T)
    wvt = np.ascontiguousarray(np.asarray(inputs["v_W"], np.float32).T)
    wft = np.ascontiguousarray(np.asarray(inputs["f_W"], np.float32).T)

    qc = _bn_consts(inputs["q_b"], inputs["q_gamma"], inputs["q_beta"],
                    inputs["q_mean"], inputs["q_var"], D1 // P)
    kc = _bn_consts(inputs["k_b"], inputs["k_gamma"], inputs["k_beta"],
                    inputs["k_mean"], inputs["k_var"], D1 // P)
    vc = _bn_consts(inputs["v_b"], inputs["v_gamma"], inputs["v_beta"],
                    inputs["v_mean"], inputs["v_var"], D2 // P)
    fc = _bn_consts(inputs["f_b"], inputs["f_gamma"], inputs["f_beta"],
                    inputs["f_mean"], inputs["f_var"], D0 // P)

    ar = np.arange(m)
    in_maps = []
    for b in range(B):
        am = np.where(ar < nvalid[b], 0.0, NEG_MASK).astype(ml_dtypes.bfloat16)
        in_maps.append({
            "x": np.ascontiguousarray(x[b]),
            "xc": np.ascontiguousarray(xc[b]),
            "wqt": wqt, "wkt": wkt, "wvt": wvt, "wft": wft,
            "amask": am,
            "qcb": qc[0], "qcg": qc[1], "qc2": qc[2],
            "kcb": kc[0], "kcg": kc[1], "kc2": kc[2],
            "vcb": vc[0], "vcg": vc[1], "vc2": vc[2],
            "fcb": fc[0], "fcg": fc[1], "fc2": fc[2],
        })

    nc = _get_program()
    res = bass_utils.run_bass_kernel_spmd(nc, in_maps, core_ids=list(range(B)),
                                          **_RUN_KWARGS)
    _CACHED["last_results"] = res
    out = np.stack([r["out"] for r in res.results], axis=0)
    return out.reshape(B, n, D0, 7, 7).astype(np.float32)


# revision 24
# speedup vs baseline: 1.0810x; 1.0810x over previous
"""Trainium2 Bass kernel for nn_Attention_60155311948227 (sparse_attention).

Sharding: data-parallel over batch B=8 across the 8 NeuronCores (1 sample per
core); the four FC weights are replicated (each core DMAs its own copy).

Per-core pipeline (GEMMs in bf16 with fp32 PSUM accumulation):
  XCT  = x_context^T   fp32 HWDGE row-strips -> PE transpose -> bf16
  A^T  = sum_{7x7}(x)  flat-layout loads, DVE reduce, PE transpose
  K^T  = BN(relu(kW @ xc^T + b))   [d1(part), m] bf16, kept in SBUF
  kn2  = ones-matmul of K^T**2 -> rk = 1/||k_row||;  qn2 -> rq
  S    = (Q^T)^T @ K^T  [n, m] * rq (row) * rk (col bcast) + amask, softmax
  P^T  = PE transpose;  P^T rows scaled by rv (V-row norms)
  V^T  -> PE transpose -> V_nat [m(part), d2] bf16 (unnormalized)
  WV^T = V_nat/P^T contraction; F^T = BN(relu(fW @ WV + b)) fp32
  out  = x + F broadcast over 7x7 (flat-layout passes, F via DRAM bounce)

Weights stream as fp32 column-strips on HWDGE and are cast to bf16 on-chip
(ACT/DVE) — the SWDGE cast-DMA path measures only ~45 GB/s aggregate.
"""

import sys

import numpy as np

try:
    import concourse.bacc as bacc
except ImportError:  # pragma: no cover
    sys.path.insert(0, "/opt/trn_rl_repo")
    import concourse.bacc as bacc

import ml_dtypes

import concourse.bass as bass
import concourse.tile as tile
from concourse import mybir
from concourse import bass_utils
from concourse.masks import make_identity

F32 = mybir.dt.float32
BF16 = mybir.dt.bfloat16
AF = mybir.ActivationFunctionType
ALU = mybir.AluOpType
AX = mybir.AxisListType

BN_EPS = 1e-5
NEG_MASK = -50.0
TEMP_INV = 100.0
NORM_EPS = 1e-24

FULL = dict(B=8, n=64, m=2048, D0=1024, C0=2048, D1=2048, D2=2048, KK=49)

P = 128


def build_program(cfg=None, num_devices=8):
    """Emit the SPMD per-core Bass program. Returns the compiled Bacc."""
    cfg = dict(FULL if cfg is None else cfg)
    n, m, D0, C0, D1, D2, KK = (
        cfg["n"], cfg["m"], cfg["D0"], cfg["C0"], cfg["D1"], cfg["D2"], cfg["KK"]
    )
    nc_d0, nc_c0, nc_d1, nc_d2, nc_m = D0 // P, C0 // P, D1 // P, D2 // P, m // P
    n_nt = max(1, m // 512)          # 512-wide moving-dim tiles
    NT = m // n_nt
    inv_kk = 1.0 / KK
    mh = m // 2
    # flat x/out chunking: partition p = (n, dhalf); per-partition contiguous
    DQ = 32                          # D-rows per flat chunk
    FD = DQ * KK                     # flat chunk free size
    NFC = (D0 // 2) // DQ            # number of flat chunks (8)

    nc = bacc.Bacc("TRN2", target_bir_lowering=False, debug=False,
                   num_devices=num_devices)

    def din(name, shape, dt=F32):
        return nc.dram_tensor(name, shape, dt, kind="ExternalInput").ap()

    x_in = din("x", [n, D0, KK])
    xc_in = din("xc", [m, C0])
    wqt = din("wqt", [D0, D1])
    wkt = din("wkt", [C0, D1])
    wvt = din("wvt", [C0, D2])
    wft = din("wft", [D2, D0])
    amask = din("amask", [m], BF16)
    qcb = din("qcb", [P, nc_d1]); qcg = din("qcg", [P, nc_d1]); qc2 = din("qc2", [P, nc_d1])
    kcb = din("kcb", [P, nc_d1]); kcg = din("kcg", [P, nc_d1]); kc2 = din("kc2", [P, nc_d1])
    vcb = din("vcb", [P, nc_d2]); vcg = din("vcg", [P, nc_d2]); vc2 = din("vc2", [P, nc_d2])
    fcb = din("fcb", [P, nc_d0]); fcg = din("fcg", [P, nc_d0]); fc2 = din("fc2", [P, nc_d0])
    out_d = nc.dram_tensor("out", [n, D0, KK], F32, kind="ExternalOutput").ap()
    x_flat = x_in.rearrange("nn d k -> (nn d k)").rearrange(
        "(p f) -> p f", p=P)          # [128, D0*KK/2] per-partition contiguous
    out_flat = out_d.rearrange("nn d k -> (nn d k)").rearrange(
        "(p f) -> p f", p=P)

    with tile.TileContext(nc) as tc:
        with (
            tc.tile_pool(name="consts", bufs=1) as consts,
            tc.tile_pool(name="bigmat", bufs=1) as bigmat,
            tc.tile_pool(name="w8", bufs=2) as w8,          # fp32 strips (8KB)
            tc.tile_pool(name="strips", bufs=2) as strips,  # bf16 strips (4KB)
            tc.tile_pool(name="smalls", bufs=2) as smalls,
            tc.tile_pool(name="wides", bufs=1) as wides,
            tc.tile_pool(name="xpool", bufs=2) as xpool,
            tc.tile_pool(name="ps", bufs=1, space="PSUM") as ps,
            tc.tile_pool(name="dscr", bufs=1, space="DRAM") as dscr,
        ):
            # ---------------- constants ----------------
            ident = consts.tile([P, P], BF16)
            make_identity(nc, ident)
            ident32 = consts.tile([P, P], F32)
            make_identity(nc, ident32)
            ones_col = consts.tile([P, 1], BF16)
            nc.vector.memset(ones_col, 1.0)
            eps_col = consts.tile([P, 1], F32)
            nc.vector.memset(eps_col, NORM_EPS)

            def cload(ap_in, nch):
                t = consts.tile([P, nch], F32, name=f"c_{ap_in.tensor.name}")
                nc.sync.dma_start(out=t, in_=ap_in)
                return t

            qcb_t = cload(qcb, nc_d1); qcg_t = cload(qcg, nc_d1); qc2_t = cload(qc2, nc_d1)
            kcb_t = cload(kcb, nc_d1); kcg_t = cload(kcg, nc_d1); kc2_t = cload(kc2, nc_d1)
            vcb_t = cload(vcb, nc_d2); vcg_t = cload(vcg, nc_d2); vc2_t = cload(vc2, nc_d2)
            fcb_t = cload(fcb, nc_d0); fcg_t = cload(fcg, nc_d0); fc2_t = cload(fc2, nc_d0)

            amask_bc = consts.tile([n, m], BF16, tag="amask_bc")
            nc.gpsimd.dma_start(
                out=amask_bc,
                in_=bass.AP(tensor=amask.tensor, offset=amask.offset,
                            ap=[[0, n]] + list(amask.ap)),
            )

            # ---------------- XCT: transpose x_context ----------------
            # contiguous fp32 row-strips; 16 fp32 PE transposes per strip into
            # an 8KB PSUM tile (alternating tag A/B); ACT copy casts to bf16.
            xct = bigmat.tile([P, nc_c0, m], BF16, tag="xct")
            for i in range(nc_m):
                xcs = w8.tile([P, C0], F32, tag="w8", name="xcs")
                nc.sync.dma_start(out=xcs, in_=xc_in[i * P:(i + 1) * P, :])
                tpx = ps.tile([P, nc_c0, P], F32,
                              tag=("A" if i % 2 == 0 else "B"), name="tpx")
                for c in range(nc_c0):
                    nc.tensor.transpose(tpx[:, c, :], xcs[:, c * P:(c + 1) * P],
                                        ident32)
                nc.scalar.copy(out=xct[:, :, i * P:(i + 1) * P], in_=tpx)

            # ---------------- pooling: A^T = sum_k x (flat layout) ----------
            at = consts.tile([P, nc_d0, n], BF16)
            for g in range(NFC):
                xt = xpool.tile([P, DQ, KK], F32, tag="x", name="xt")
                nc.sync.dma_start(out=xt,
                                  in_=x_flat[:, g * FD:(g + 1) * FD])
                asum = smalls.tile([P, DQ], F32, name="asum")
                nc.vector.reduce_sum(asum, xt, axis=AX.X)
                atp = ps.tile([DQ, P], F32, tag="B", name="atp")
                nc.tensor.transpose(atp, asum, ident32)
                # columns p=(nn, dhalf); D row = dhalf*D0/2 + g*DQ + dd2
                for half in range(2):
                    dglob = half * (D0 // 2) + g * DQ
                    base = dglob % P
                    nc.vector.tensor_copy(
                        out=at[base:base + DQ, dglob // P, :],
                        in_=atp[:, half::2])

            # ---------------- K^T projection (kept in SBUF) ----------------
            def wstrip(w_ap, j, ncc, name):
                """fp32 column-strip [P, ncc, P] -> cast to bf16 on-chip."""
                wf = w8.tile([P, ncc, P], F32, tag="w8", name=f"{name}f")
                nc.sync.dma_start(
                    out=wf,
                    in_=w_ap[:, j * P:(j + 1) * P].rearrange(
                        "(c p) w -> p c w", p=P))
                wb = strips.tile([P, ncc, P], BF16, tag="strip", name=f"{name}b")
                if j % 2 == 0:
                    nc.vector.tensor_copy(out=wb, in_=wf)
                else:
                    nc.scalar.copy(out=wb, in_=wf)
                return wb

            kt = bigmat.tile([P, nc_d1, m], BF16, tag="ktv", name="kt")
            kn2 = ps.tile([1, m], F32, tag="B")
            for j in range(nc_d1):
                kws = wstrip(wkt, j, nc_c0, "kws")
                kp = ps.tile([P, m], F32, tag="A", name="kp")
                for c in range(nc_c0):
                    for nt in range(n_nt):
                        nc.tensor.matmul(kp[:, nt * NT:(nt + 1) * NT],
                                         kws[:, c, :],
                                         xct[:, c, nt * NT:(nt + 1) * NT],
                                         start=(c == 0), stop=(c == nc_c0 - 1))
                ktj = kt[:, j, :]
                nc.scalar.activation(ktj[:, :mh], kp[:, :mh], AF.Relu,
                                     bias=kcb_t[:, j:j + 1])
                nc.vector.tensor_scalar(out=ktj[:, mh:], in0=kp[:, mh:],
                                        scalar1=kcb_t[:, j:j + 1], scalar2=0.0,
                                        op0=ALU.add, op1=ALU.max)
                nc.vector.tensor_scalar(out=ktj, in0=ktj,
                                        scalar1=kcg_t[:, j:j + 1],
                                        scalar2=kc2_t[:, j:j + 1],
                                        op0=ALU.mult, op1=ALU.add)
                ksq = w8.tile([P, m], BF16, tag="w8", name="ksq")
                nc.scalar.activation(ksq, ktj, AF.Square)
                for nt in range(n_nt):
                    nc.tensor.matmul(kn2[:, nt * NT:(nt + 1) * NT], ones_col,
                                     ksq[:, nt * NT:(nt + 1) * NT],
                                     start=(j == 0), stop=(j == nc_d1 - 1))
            # rk chain: sqrt -> scatter [P, m/P] -> recip -> DRAM -> bcast
            rk_row = smalls.tile([1, m], F32, name="rk_row")
            nc.scalar.activation(rk_row, kn2, AF.Sqrt, bias=eps_col[:1, :])
            scr_k = dscr.tile([m], F32, name="scr_k")
            nc.gpsimd.dma_start(out=scr_k, in_=rk_row)
            rk128 = smalls.tile([P, nc_m], F32, name="rk128")
            nc.gpsimd.dma_start(out=rk128,
                                in_=bass.AP(tensor=scr_k.tensor, offset=scr_k.offset,
                                            ap=[[1, P], [P, nc_m]]))
            nc.vector.reciprocal(rk128, rk128)
            scr_k2 = dscr.tile([m], F32, name="scr_k2")
            nc.gpsimd.dma_start(
                out=bass.AP(tensor=scr_k2.tensor, offset=scr_k2.offset,
                            ap=[[1, P], [P, nc_m]]),
                in_=rk128)
            rk_bc = wides.tile([n, m], F32, name="rk_bc", tag="rk_bc")
            nc.gpsimd.dma_start(out=rk_bc,
                                in_=bass.AP(tensor=scr_k2.tensor, offset=scr_k2.offset,
                                            ap=[[0, n], [1, m]]))

            # ---------------- Q^T projection ----------------
            qt = consts.tile([P, nc_d1, n], BF16)
            qn2 = ps.tile([1, n], F32, tag="B")
            for j in range(nc_d1):
                qws = wstrip(wqt, j, nc_d0, "qws")
                qp = ps.tile([P, n], F32, tag="A", name="qp")
                for c in range(nc_d0):
                    nc.tensor.matmul(qp, qws[:, c, :], at[:, c, :],
                                     start=(c == 0), stop=(c == nc_d0 - 1))
                q1 = smalls.tile([P, n], BF16, name="q1")
                nc.scalar.activation(q1, qp, AF.Relu, bias=qcb_t[:, j:j + 1],
                                     scale=inv_kk)
                nc.vector.tensor_scalar(out=qt[:, j, :], in0=q1,
                                        scalar1=qcg_t[:, j:j + 1],
                                        scalar2=qc2_t[:, j:j + 1],
                                        op0=ALU.mult, op1=ALU.add)
                qsq = smalls.tile([P, n], BF16, name="qsq")
                nc.scalar.activation(qsq, qt[:, j, :], AF.Square)
                nc.tensor.matmul(qn2, ones_col, qsq,
                                 start=(j == 0), stop=(j == nc_d1 - 1))
            rq_row = smalls.tile([1, n], F32, name="rq_row")
            nc.scalar.activation(rq_row, qn2, AF.Sqrt, bias=eps_col[:1, :])
            scr_q = dscr.tile([n], F32, name="scr_q")
            nc.gpsimd.dma_start(out=scr_q, in_=rq_row)
            rq_col = smalls.tile([n, 1], F32, name="rq_col")
            nc.gpsimd.dma_start(out=rq_col,
                                in_=bass.AP(tensor=scr_q.tensor, offset=scr_q.offset,
                                            ap=[[1, n], [1, 1]]))
            nc.vector.reciprocal(rq_col, rq_col)

            # ---------------- S = Q K^T, softmax ----------------
            sp = ps.tile([n, m], F32, tag="B", name="sp")
            for j in range(nc_d1):
                for nt in range(n_nt):
                    nc.tensor.matmul(sp[:, nt * NT:(nt + 1) * NT], qt[:, j, :],
                                     kt[:, j, nt * NT:(nt + 1) * NT],
                                     start=(j == 0), stop=(j == nc_d1 - 1))
            nc.vector.tensor_scalar(out=sp, in0=sp, scalar1=rq_col,
                                    scalar2=None, op0=ALU.mult)
            nc.vector.tensor_mul(sp, sp, rk_bc)
            nc.vector.tensor_add(sp, sp, amask_bc)
            mxn = smalls.tile([n, 1], F32, name="mxn")
            nc.vector.tensor_reduce(mxn, sp, axis=AX.X, op=ALU.max, negate=True)
            ebias = smalls.tile([n, 1], F32, name="ebias")
            nc.vector.tensor_scalar_mul(ebias, mxn, TEMP_INV)
            p_t = consts.tile([n, m], BF16, name="p_t", tag="amask_bc")
            pden = smalls.tile([n, 1], F32, name="pden")
            nc.scalar.activation(p_t, sp, AF.Exp, bias=ebias, scale=TEMP_INV,
                                 accum_out=pden)
            nc.vector.reciprocal(pden, pden)
            nc.vector.tensor_scalar_mul(p_t, p_t, pden)
            ptp = ps.tile([P, nc_m, n], BF16, tag="B", name="ptp")
            for i in range(nc_m):
                nc.tensor.transpose(ptp[:, i, :], p_t[:, i * P:(i + 1) * P],
                                    ident[:n, :n])
            pt_sb = consts.tile([P, nc_m, n], BF16)
            nc.vector.tensor_copy(out=pt_sb, in_=ptp)

            # ---------------- V^T -> V_nat (unnormalized) ----------------
            v_nat = bigmat.tile([P, nc_m, D2], BF16, tag="ktv", name="v_nat")
            for j in range(nc_d2):
                vws = wstrip(wvt, j, nc_c0, "vws")
                vp = ps.tile([P, m], F32, tag="A", name="vp")
                for c in range(nc_c0):
                    for nt in range(n_nt):
                        nc.tensor.matmul(vp[:, nt * NT:(nt + 1) * NT],
                                         vws[:, c, :],
                                         xct[:, c, nt * NT:(nt + 1) * NT],
                                         start=(c == 0), stop=(c == nc_c0 - 1))
                vtj = strips.tile([P, m], BF16, tag="strip", name="vtj")
                nc.scalar.activation(vtj[:, :mh], vp[:, :mh], AF.Relu,
                                     bias=vcb_t[:, j:j + 1])
                nc.vector.tensor_scalar(out=vtj[:, mh:], in0=vp[:, mh:],
                                        scalar1=vcb_t[:, j:j + 1], scalar2=0.0,
                                        op0=ALU.add, op1=ALU.max)
                nc.vector.tensor_scalar(out=vtj, in0=vtj,
                                        scalar1=vcg_t[:, j:j + 1],
                                        scalar2=vc2_t[:, j:j + 1],
                                        op0=ALU.mult, op1=ALU.add)
                vtp = ps.tile([P, nc_m, P], BF16, tag="B", name="vtp")
                for i in range(nc_m):
                    nc.tensor.transpose(vtp[:, i, :], vtj[:, i * P:(i + 1) * P],
                                        ident)
                nc.vector.tensor_copy(out=v_nat[:, :, j * P:(j + 1) * P],
                                      in_=vtp)
            # rv = 1/||v_row||; folded into P^T rows (per-partition there)
            for i in range(nc_m):
                vsq = w8.tile([P, D2], BF16, tag="w8", name="vsq")
                vn2 = smalls.tile([P, 1], F32, name="vn2")
                nc.scalar.activation(vsq, v_nat[:, i, :], AF.Square,
                                     accum_out=vn2)
                rv = smalls.tile([P, 1], F32, name="rv")
                nc.scalar.activation(rv, vn2, AF.Sqrt, bias=eps_col)
                nc.vector.reciprocal(rv, rv)
                nc.vector.tensor_scalar_mul(pt_sb[:, i, :], pt_sb[:, i, :], rv)

            # ---------------- WV^T = sum_i V_nat_i^T P^T_i ----------------
            wvt_sb = consts.tile([P, nc_d2, n], BF16)
            for j in range(nc_d2):
                wvp = ps.tile([P, n], F32, tag="A", name="wvp")
                for i in range(nc_m):
                    nc.tensor.matmul(wvp, v_nat[:, i, j * P:(j + 1) * P],
                                     pt_sb[:, i, :],
                                     start=(i == 0), stop=(i == nc_m - 1))
                nc.vector.tensor_copy(out=wvt_sb[:, j, :], in_=wvp)

            # ---------------- F^T projection (fp32) ----------------
            ft = consts.tile([P, nc_d0, n], F32)
            for dd in range(nc_d0):
                fws = wstrip(wft, dd, nc_d2, "fws")
                fp = ps.tile([P, n], F32, tag="A", name="fp")
                for j in range(nc_d2):
                    nc.tensor.matmul(fp, fws[:, j, :], wvt_sb[:, j, :],
                                     start=(j == 0), stop=(j == nc_d2 - 1))
                f1 = smalls.tile([P, n], F32, name="f1")
                nc.scalar.activation(f1, fp, AF.Relu, bias=fcb_t[:, dd:dd + 1])
                nc.vector.tensor_scalar(out=ft[:, dd, :], in0=f1,
                                        scalar1=fcg_t[:, dd:dd + 1],
                                        scalar2=fc2_t[:, dd:dd + 1],
                                        op0=ALU.mult, op1=ALU.add)

            # ---------------- out = x + F (flat layout) ----------------
            # F^T -> F_nat (PE transposes) -> DRAM bounce -> [(n dhalf), D0/2]
            fnat = wides.tile([n, D0], F32, tag="rk_bc")
            for dd in range(nc_d0):
                ftp = ps.tile([n, P], F32, tag="B", name="ftp")
                nc.tensor.transpose(ftp, ft[:, dd, :], ident32)
                nc.vector.tensor_copy(out=fnat[:, dd * P:(dd + 1) * P], in_=ftp)
            f_scr = dscr.tile([n, D0], F32, name="f_scr")
            nc.sync.dma_start(out=f_scr, in_=fnat)
            fperm = wides.tile([P, D0 // 2], F32, name="fperm", tag="rk_bc")
            nc.sync.dma_start(
                out=fperm,
                in_=bass.AP(tensor=f_scr.tensor, offset=f_scr.offset,
                            ap=[[D0, n], [D0 // 2, 2], [1, D0 // 2]]))
            for g in range(NFC):
                xo = xpool.tile([P, DQ, KK], F32, tag="x", name="xo")
                nc.sync.dma_start(out=xo, in_=x_flat[:, g * FD:(g + 1) * FD])
                nc.vector.tensor_add(
                    xo, xo,
                    fperm[:, g * DQ:(g + 1) * DQ].unsqueeze(2)
                    .broadcast_to([P, DQ, KK]))
                nc.scalar.dma_start(out=out_flat[:, g * FD:(g + 1) * FD], in_=xo)

    nc.compile()
    return nc


_CACHED = {}
# test-harness hook: extra kwargs for run_bass_kernel_spmd (e.g. trace=True)
_RUN_KWARGS = {}


def _get_program():
    if "nc" not in _CACHED:
        _CACHED["nc"] = build_program()
    return _CACHED["nc"]


def _bn_consts(b, gamma, beta, mean, var, nch):
    g = (gamma / np.sqrt(var + BN_EPS)).astype(np.float32)
    b2 = (beta - g * mean).astype(np.float32)
    def fold(v):
        return np.ascontiguousarray(np.asarray(v, np.float32).reshape(nch, P).T)
    return fold(b), fold(g), fold(b2)


def kernel(**inputs):
    cfg = FULL
    B, n, m = cfg["B"], cfg["n"], cfg["m"]
    D0, C0, D1, D2, KK = cfg["D0"], cfg["C0"], cfg["D1"], cfg["D2"], cfg["KK"]

    x = np.asarray(inputs["x"], dtype=np.float32).reshape(B, n, D0, KK)
    xc = np.asarray(inputs["x_context"], dtype=np.float32)
    nvalid = np.asarray(inputs["num_valid_context_items"]).reshape(B).astype(np.int64)

    wqt = np.ascontiguousarray(np.asarray(inputs["q_W"], np.float32).T)
    wkt = np.ascontiguousarray(np.asarray(inputs["k_W"], np.float32).T)
    wvt = np.ascontiguousarray(np.asarray(inputs["v_W"], np.float32).T)
    wft = np.ascontiguousarray(np.asarray(inputs["f_W"], np.float32).T)

    qc = _bn_consts(inputs["q_b"], inputs["q_gamma"], inputs["q_beta"],
                    inputs["q_mean"], inputs["q_var"], D1 // P)
    kc = _bn_consts(inputs["k_b"], inputs["k_gamma"], inputs["k_beta"],
                    inputs["k_mean"], inputs["k_var"], D1 // P)
    vc = _bn_consts(inputs["v_b"], inputs["v_gamma"], inputs["v_beta"],
                    inputs["v_mean"], inputs["v_var"], D2 // P)
    fc = _bn_consts(inputs["f_b"], inputs["f_gamma"], inputs["f_beta"],
                    inputs["f_mean"], inputs["f_var"], D0 // P)

    ar = np.arange(m)
    in_maps = []
    for b in range(B):
        am = np.where(ar < nvalid[b], 0.0, NEG_MASK).astype(ml_dtypes.bfloat16)
        in_maps.append({
            "x": np.ascontiguousarray(x[b]),
            "xc": np.ascontiguousarray(xc[b]),
            "wqt": wqt, "wkt": wkt, "wvt": wvt, "wft": wft,
            "amask": am,
            "qcb": qc[0], "qcg": qc[1], "qc2": qc[2],
            "kcb": kc[0], "kcg": kc[1], "kc2": kc[2],
            "vcb": vc[0], "vcg": vc[1], "vc2": vc[2],
            "fcb": fc[0], "fcg": fc[1], "fc2": fc[2],
        })

    nc = _get_program()
    res = bass_utils.run_bass_kernel_spmd(nc, in_maps, core_ids=list(range(B)),
                                          **_RUN_KWARGS)
    _CACHED["last_results"] = res
    out = np.stack([r["out"] for r in res.results], axis=0)
    return out.reshape(B, n, D0, 7, 7).astype(np.float32)
